# revision 12
# baseline (speedup 1.0000x reference)
"""Trainium2 Bass kernel for 2-layer GAT (nn_GAT_30382598652184).

Strategy (8 NeuronCores, SPMD):
  - Row-shard the N=8192 attention rows: core k owns rows [k*1024, (k+1)*1024).
  - Each core computes its rows' e/softmax/aggregation in a transposed layout:
    j (attention source node) on SBUF partitions (64 chunks of 128), the core's
    1024 rows on the free dim.
  - e_ij = leakyrelu(src_i + dst_j) with adjacency mask folded in additively on
    the host: adj is pre-transformed to fp16 {0, -100} (scaled by 0.4) so that
    masked entries produce exp(~-50) -> 0 exactly in fp16.
  - leakyrelu(s) = 0.6*s + 0.4*|s| is computed as t = 1.5*s4 + |s4| on the
    pre-scaled s4 = 0.4*s, using one scalar_tensor_tensor + one tensor_scalar
    (abs_max) + one scalar_tensor_tensor on the vector engine; exp on ScalarE.
  - Aggregation att@Wh and the softmax denominator come from a single PE
    accumulation against Whx = [Wh | 1] (ones column -> row sums).
  - One AllGather (x2 transposed shards) between the two GAT layers.
All sharding/shapes are hardcoded; inputs arrive full and the full output is
reassembled on the host.
"""

import numpy as np

import concourse.bass as bass
import concourse.bacc as bacc
import concourse.mybir as mybir
import concourse.tile as tile
from concourse.bass_utils import run_bass_kernel_spmd

N = 8192
NU = 4096
D = 64
NCORES = 8
R = N // NCORES  # 1024 rows per core
NCH = N // 128  # 64 chunks of 128 source nodes
F16 = mybir.dt.float16
F32 = mybir.dt.float32
AOP = mybir.AluOpType
AF = mybir.ActivationFunctionType
MASK0 = -100.0  # pre-scaled (0.4x) additive mask for adj==0


def _build_bass():
    nc = bacc.Bacc(num_devices=NCORES)

    adjm = nc.dram_tensor("adjm", [N, R], F16, kind="ExternalInput")
    xTa = nc.dram_tensor("xTa", [D + 1, N], F32, kind="ExternalInput")
    xTm = nc.dram_tensor("xTm", [D + 1, R], F32, kind="ExternalInput")
    w0tb = nc.dram_tensor("w0tb", [D + 1, D], F32, kind="ExternalInput")
    w1tb = nc.dram_tensor("w1tb", [D + 1, D], F32, kind="ExternalInput")
    wsrc0 = nc.dram_tensor("wsrc0", [D + 1, 1], F32, kind="ExternalInput")
    wdst0 = nc.dram_tensor("wdst0", [D + 1, 1], F32, kind="ExternalInput")
    wsrc1 = nc.dram_tensor("wsrc1", [D + 1, 1], F32, kind="ExternalInput")
    wdst1 = nc.dram_tensor("wdst1", [D + 1, 1], F32, kind="ExternalInput")
    owt = nc.dram_tensor("owt", [D, D], F32, kind="ExternalInput")
    outb = nc.dram_tensor("outb", [D, 1], F32, kind="ExternalInput")
    outT = nc.dram_tensor("outT", [D, R], F32, kind="ExternalOutput")

    with tile.TileContext(nc) as tc:
        with (
            tc.tile_pool(name="const", bufs=1) as const,
            tc.tile_pool(name="perlayer", bufs=2) as perlayer,
            tc.tile_pool(name="work", bufs=4) as work,
            tc.tile_pool(name="psA", bufs=2, space="PSUM") as psA,
            tc.tile_pool(name="psB", bufs=2, space="PSUM") as psB,
            tc.tile_pool(name="dram", bufs=1, space="DRAM") as dram,
        ):
            # ---- load constants ----
            xTa_sb = const.tile([D + 1, N], F32, tag="xTa")
            nc.sync.dma_start(xTa_sb[:], xTa[:])
            xTm_sb = const.tile([D + 1, R], F32, tag="xTm")
            nc.sync.dma_start(xTm_sb[:], xTm[:])
            w0tb_sb = const.tile([D + 1, D], F32, tag="w0tb")
            nc.sync.dma_start(w0tb_sb[:], w0tb[:])
            w1tb_sb = const.tile([D + 1, D], F32, tag="w1tb")
            nc.sync.dma_start(w1tb_sb[:], w1tb[:])
            wsrc0_sb = const.tile([D + 1, 1], F32, tag="wsrc0")
            nc.sync.dma_start(wsrc0_sb[:], wsrc0[:])
            wdst0_sb = const.tile([D + 1, 1], F32, tag="wdst0")
            nc.sync.dma_start(wdst0_sb[:], wdst0[:])
            wsrc1_sb = const.tile([D + 1, 1], F32, tag="wsrc1")
            nc.sync.dma_start(wsrc1_sb[:], wsrc1[:])
            wdst1_sb = const.tile([D + 1, 1], F32, tag="wdst1")
            nc.sync.dma_start(wdst1_sb[:], wdst1[:])
            owt_sb = const.tile([D, D], F32, tag="owt")
            nc.sync.dma_start(owt_sb[:], owt[:])
            outb_sb = const.tile([D, 1], F32, tag="outb")
            nc.sync.dma_start(outb_sb[:], outb[:])
            ones128 = const.tile([1, 128], F32, tag="ones128")
            nc.vector.memset(ones128[:], 1.0)

            def gat_layer(xa_sb, xm_sb, wtb_sb, wsrc_sb, wdst_sb):
                """One GAT layer. xa_sb: [65, 8192] augmented x.T for all nodes;
                xm_sb: [65, 1024] augmented x.T for this core's rows.
                Returns xnT [65, 1024] f32 tile = relu(att@Wh).T (row 64 = ones).
                """
                # Wh chunks in [j, d] layout (+ ones column) for the aggregation
                whx = perlayer.tile([128, NCH * (D + 1)], F16, tag="whx")
                nc.vector.memset(whx[:], 1.0)
                whx3 = whx.rearrange("p (c w) -> p c w", w=D + 1)
                for g in range(8):
                    ps = psB.tile([128, 512], F32, tag="psB")
                    for cc in range(8):
                        c = g * 8 + cc
                        nc.tensor.matmul(
                            ps[:, cc * D : (cc + 1) * D],
                            lhsT=xa_sb[:, c * 128 : (c + 1) * 128],
                            rhs=wtb_sb[:],
                            start=True,
                            stop=True,
                        )
                    nc.scalar.activation(
                        whx3[:, g * 8 : (g + 1) * 8, 0:D], ps[:], AF.Copy
                    )

                # dst contribution, per-partition per chunk: [128, 64] f32
                dstc = perlayer.tile([128, NCH], F32, tag="dstc")
                for g in range(8):
                    psd = psB.tile([128, 8], F32, tag="psD")
                    for cc in range(8):
                        c = g * 8 + cc
                        nc.tensor.matmul(
                            psd[:, cc : cc + 1],
                            lhsT=xa_sb[:, c * 128 : (c + 1) * 128],
                            rhs=wdst_sb[:],
                            start=True,
                            stop=True,
                        )
                    nc.scalar.activation(dstc[:, g * 8 : (g + 1) * 8], psd[:], AF.Copy)

                # src contribution for this core's rows: [1, 1024] -> bcast
                srcf = perlayer.tile([1, R], F32, tag="srcf")
                for h in range(2):
                    pss = psB.tile([1, 512], F32, tag="psB")
                    nc.tensor.matmul(
                        pss[:],
                        lhsT=wsrc_sb[:],
                        rhs=xm_sb[:, h * 512 : (h + 1) * 512],
                        start=True,
                        stop=True,
                    )
                    nc.scalar.activation(
                        srcf[:, h * 512 : (h + 1) * 512], pss[:], AF.Copy
                    )
                srcrep = perlayer.tile([128, R], F16, tag="srcrep")
                for h in range(2):
                    psb = psB.tile([128, 512], F32, tag="psB")
                    nc.tensor.matmul(
                        psb[:], lhsT=ones128[:], rhs=srcf[:, h * 512 : (h + 1) * 512],
                        start=True, stop=True,
                    )
                    nc.scalar.activation(
                        srcrep[:, h * 512 : (h + 1) * 512], psb[:], AF.Copy
                    )

                # main loop over the 64 source-node chunks
                agg0 = psA.tile([D + 1, 512], F32, tag="agg0")
                agg1 = psA.tile([D + 1, 512], F32, tag="agg1")
                for c in range(NCH):
                    adjc = work.tile([128, R], F16, tag="adj")
                    nc.sync.dma_start(adjc[:], adjm[c * 128 : (c + 1) * 128, :])
                    s4 = work.tile([128, R], F16, tag="s4")
                    nc.vector.scalar_tensor_tensor(
                        s4[:], adjc[:], dstc[:, c : c + 1], srcrep[:],
                        op0=AOP.add, op1=AOP.add,
                    )
                    av = work.tile([128, R], F16, tag="av")
                    nc.vector.tensor_scalar(
                        av[:], s4[:], 0.0, 2.0, op0=AOP.max, op1=AOP.mult
                    )
                    tv = work.tile([128, R], F16, tag="tv")
                    nc.vector.scalar_tensor_tensor(
                        tv[:], s4[:], 0.5, av[:], op0=AOP.mult, op1=AOP.add
                    )
                    pv = work.tile([128, R], F16, tag="pv")
                    nc.scalar.activation(pv[:], tv[:], AF.Exp)
                    nc.tensor.matmul(
                        agg0[:], lhsT=whx3[:, c, :], rhs=pv[:, 0:512],
                        start=(c == 0), stop=(c == NCH - 1),
                    )
                    nc.tensor.matmul(
                        agg1[:], lhsT=whx3[:, c, :], rhs=pv[:, 512:1024],
                        start=(c == 0), stop=(c == NCH - 1),
                    )

                # normalize + relu -> xnT [65, 1024] (row 64 = ones)
                zinv = perlayer.tile([1, R], F32, tag="zinv")
                nc.vector.reciprocal(zinv[:, 0:512], agg0[D : D + 1, :])
                nc.vector.reciprocal(zinv[:, 512:1024], agg1[D : D + 1, :])
                zrep = perlayer.tile([D, R], F32, tag="zrep")
                for h in range(2):
                    psb = psB.tile([D, 512], F32, tag="psB")
                    nc.tensor.matmul(
                        psb[:], lhsT=ones128[:, 0:D],
                        rhs=zinv[:, h * 512 : (h + 1) * 512],
                        start=True, stop=True,
                    )
                    nc.scalar.activation(
                        zrep[:, h * 512 : (h + 1) * 512], psb[:], AF.Copy
                    )
                xnT = perlayer.tile([D + 1, R], F32, tag="xnT")
                nc.vector.memset(xnT[D : D + 1, :], 1.0)
                nc.vector.tensor_tensor(
                    xnT[0:D, 0:512], agg0[0:D, :], zrep[:, 0:512], AOP.mult
                )
                nc.vector.tensor_tensor(
                    xnT[0:D, 512:1024], agg1[0:D, :], zrep[:, 512:1024], AOP.mult
                )
                nc.scalar.activation(xnT[0:D, :], xnT[0:D, :], AF.Relu)
                return xnT

            # ---------------- layer 0 ----------------
            x1T = gat_layer(xTa_sb, xTm_sb, w0tb_sb, wsrc0_sb, wdst0_sb)

            # AllGather x1 shards (transposed) across the 8 cores
            bounce = dram.tile([D, R], F32)
            nc.sync.dma_start(bounce[:], x1T[0:D, :])
            gath = dram.tile([NCORES * D, R], F32, addr_space="Shared")
            nc.gpsimd.collective_compute(
                "AllGather",
                AOP.bypass,
                replica_groups=[list(range(NCORES))],
                ins=[bounce[:]],
                outs=[gath[:]],
            )
            x1g = perlayer.tile([D + 1, N], F32, tag="xg")
            nc.vector.memset(x1g[D : D + 1, :], 1.0)
            for b in range(NCORES):
                nc.sync.dma_start(
                    x1g[0:D, b * R : (b + 1) * R], gath[b * D : (b + 1) * D, :]
                )

            # ---------------- layer 1 ----------------
            x2T = gat_layer(x1g, x1T, w1tb_sb, wsrc1_sb, wdst1_sb)

            # ---------------- output linear ----------------
            outsb = const.tile([D, R], F32, tag="outsb")
            for h in range(2):
                psf = psB.tile([D, 512], F32, tag="psB")
                nc.tensor.matmul(
                    psf[:],
                    lhsT=owt_sb[:],
                    rhs=x2T[0:D, h * 512 : (h + 1) * 512],
                    start=True,
                    stop=True,
                )
                nc.scalar.activation(
                    outsb[:, h * 512 : (h + 1) * 512], psf[:], AF.Identity,
                    bias=outb_sb[:, 0:1],
                )
            nc.sync.dma_start(outT[:], outsb[:])

    nc.compile()
    return nc


def _prep_inputs(adj, user_emb, item_emb, W0_w, W0_b, a0, W1_w, W1_b, a1,
                 out_w, out_b):
    x = np.concatenate([np.asarray(user_emb), np.asarray(item_emb)], axis=0)
    x = x.astype(np.float32)
    xTa = np.concatenate([x.T, np.ones((1, N), np.float32)], axis=0)
    xTa = np.ascontiguousarray(xTa)

    adj = np.asarray(adj)
    adjm_full = ((adj - 1) * 100).astype(np.float16)  # {0, -100}, 0.4-pre-scaled

    def aug_wt(W, b):
        return np.ascontiguousarray(
            np.concatenate([W.T, b[None, :]], axis=0).astype(np.float32)
        )

    def aug_attn(W, b, avec):
        w = W.T.astype(np.float64) @ avec.astype(np.float64).reshape(D, 1)
        c = float(b.astype(np.float64) @ avec.astype(np.float64).reshape(D))
        v = np.concatenate([w, [[c]]], axis=0) * 0.4
        return np.ascontiguousarray(v.astype(np.float32))

    W0_w, W0_b = np.asarray(W0_w, np.float32), np.asarray(W0_b, np.float32)
    W1_w, W1_b = np.asarray(W1_w, np.float32), np.asarray(W1_b, np.float32)
    a0, a1 = np.asarray(a0, np.float32), np.asarray(a1, np.float32)
    out_w, out_b = np.asarray(out_w, np.float32), np.asarray(out_b, np.float32)

    shared = {
        "xTa": xTa,
        "w0tb": aug_wt(W0_w, W0_b),
        "w1tb": aug_wt(W1_w, W1_b),
        "wsrc0": aug_attn(W0_w, W0_b, a0[:D]),
        "wdst0": aug_attn(W0_w, W0_b, a0[D:]),
        "wsrc1": aug_attn(W1_w, W1_b, a1[:D]),
        "wdst1": aug_attn(W1_w, W1_b, a1[D:]),
        "owt": np.ascontiguousarray(out_w.T.astype(np.float32)),
        "outb": np.ascontiguousarray(out_b.reshape(D, 1).astype(np.float32)),
    }
    in_maps = []
    for k in range(NCORES):
        m = dict(shared)
        m["adjm"] = np.ascontiguousarray(adjm_full[k * R : (k + 1) * R, :].T)
        m["xTm"] = np.ascontiguousarray(xTa[:, k * R : (k + 1) * R])
        in_maps.append(m)
    return in_maps


_NC_CACHE = {}


def run(inputs: dict, trace: bool = False):
    if "nc" not in _NC_CACHE:
        _NC_CACHE["nc"] = _build_bass()
    nc = _NC_CACHE["nc"]
    in_maps = _prep_inputs(**inputs)
    res = run_bass_kernel_spmd(nc, in_maps, list(range(NCORES)), trace=trace)
    shards = [res.results[k]["outT"].T for k in range(NCORES)]
    full = np.concatenate(shards, axis=0).astype(np.float32)
    return (full[:NU], full[NU:]), res


def kernel(**inputs):
    out, _ = run(inputs, trace=False)
    return out


# revision 16
# speedup vs baseline: 1.1662x; 1.1662x over previous
"""Trainium2 Bass kernel for 2-layer GAT (nn_GAT_30382598652184).

Strategy (8 NeuronCores, SPMD):
  - Row-shard the N=8192 attention rows: core k owns rows [k*1024, (k+1)*1024).
  - Each core computes its rows' e/softmax/aggregation in a transposed layout:
    j (attention source node) on SBUF partitions (64 chunks of 128), the core's
    1024 rows on the free dim.
  - e_ij = leakyrelu(src_i + dst_j) with adjacency mask folded in additively on
    the host: adj is pre-transformed to fp16 {0, -100} (scaled by 0.4) so that
    masked entries produce exp(~-50) -> 0 exactly in fp16.
  - leakyrelu(s) = 0.6*s + 0.4*|s| is computed as t = 1.5*s4 + |s4| on the
    pre-scaled s4 = 0.4*s, using one scalar_tensor_tensor + one tensor_scalar
    (abs_max) + one scalar_tensor_tensor on the vector engine; exp on ScalarE.
  - Aggregation att@Wh and the softmax denominator come from a single PE
    accumulation against Whx = [Wh | 1] (ones column -> row sums).
  - One AllGather (x2 transposed shards) between the two GAT layers.
All sharding/shapes are hardcoded; inputs arrive full and the full output is
reassembled on the host.
"""

import numpy as np

import concourse.bass as bass
import concourse.bacc as bacc
import concourse.mybir as mybir
import concourse.tile as tile
from concourse.bass_utils import run_bass_kernel_spmd

N = 8192
NU = 4096
D = 64
NCORES = 8
R = N // NCORES  # 1024 rows per core
NCH = N // 128  # 64 chunks of 128 source nodes
F16 = mybir.dt.float16
F32 = mybir.dt.float32
AOP = mybir.AluOpType
AF = mybir.ActivationFunctionType
MASK0 = -100.0  # pre-scaled (0.4x) additive mask for adj==0
GPSIMD_TT_FRAC4 = 0  # of every 4 chunks, how many route the final TT to GpSimd


def _build_bass():
    nc = bacc.Bacc(num_devices=NCORES)

    adjm = nc.dram_tensor("adjm", [N, R], F16, kind="ExternalInput")
    xTa = nc.dram_tensor("xTa", [D + 1, N], F32, kind="ExternalInput")
    xTm = nc.dram_tensor("xTm", [D + 1, R], F32, kind="ExternalInput")
    w0tb = nc.dram_tensor("w0tb", [D + 1, D], F32, kind="ExternalInput")
    w1tb = nc.dram_tensor("w1tb", [D + 1, D], F32, kind="ExternalInput")
    wsrc0 = nc.dram_tensor("wsrc0", [D + 1, 1], F32, kind="ExternalInput")
    wdst0 = nc.dram_tensor("wdst0", [D + 1, 1], F32, kind="ExternalInput")
    wsrc1 = nc.dram_tensor("wsrc1", [D + 1, 1], F32, kind="ExternalInput")
    wdst1 = nc.dram_tensor("wdst1", [D + 1, 1], F32, kind="ExternalInput")
    owt = nc.dram_tensor("owt", [D, D], F32, kind="ExternalInput")
    outb = nc.dram_tensor("outb", [D, 1], F32, kind="ExternalInput")
    outT = nc.dram_tensor("outT", [D, R], F32, kind="ExternalOutput")

    with tile.TileContext(nc) as tc:
        with (
            tc.tile_pool(name="const", bufs=1) as const,
            tc.tile_pool(name="perlayer", bufs=2) as perlayer,
            tc.tile_pool(name="work", bufs=2) as work,
            tc.tile_pool(name="psA", bufs=2, space="PSUM") as psA,
            tc.tile_pool(name="psB", bufs=2, space="PSUM") as psB,
            tc.tile_pool(name="dram", bufs=1, space="DRAM") as dram,
        ):
            # ---- load constants ----
            xTa_sb = const.tile([D + 1, N], F32, tag="xTa")
            nc.sync.dma_start(xTa_sb[:], xTa[:])
            xTm_sb = const.tile([D + 1, R], F32, tag="xTm")
            nc.sync.dma_start(xTm_sb[:], xTm[:])
            w0tb_sb = const.tile([D + 1, D], F32, tag="w0tb")
            nc.sync.dma_start(w0tb_sb[:], w0tb[:])
            w1tb_sb = const.tile([D + 1, D], F32, tag="w1tb")
            nc.sync.dma_start(w1tb_sb[:], w1tb[:])
            wsrc0_sb = const.tile([D + 1, 1], F32, tag="wsrc0")
            nc.sync.dma_start(wsrc0_sb[:], wsrc0[:])
            wdst0_sb = const.tile([D + 1, 1], F32, tag="wdst0")
            nc.sync.dma_start(wdst0_sb[:], wdst0[:])
            wsrc1_sb = const.tile([D + 1, 1], F32, tag="wsrc1")
            nc.sync.dma_start(wsrc1_sb[:], wsrc1[:])
            wdst1_sb = const.tile([D + 1, 1], F32, tag="wdst1")
            nc.sync.dma_start(wdst1_sb[:], wdst1[:])
            owt_sb = const.tile([D, D], F32, tag="owt")
            nc.sync.dma_start(owt_sb[:], owt[:])
            outb_sb = const.tile([D, 1], F32, tag="outb")
            nc.sync.dma_start(outb_sb[:], outb[:])
            ones128 = const.tile([1, 128], F32, tag="ones128")
            nc.vector.memset(ones128[:], 1.0)

            def gat_layer(xa_sb, xm_sb, wtb_sb, wsrc_sb, wdst_sb):
                """One GAT layer. xa_sb: [65, 8192] augmented x.T for all nodes;
                xm_sb: [65, 1024] augmented x.T for this core's rows.
                Returns xnT [65, 1024] f32 tile = relu(att@Wh).T (row 64 = ones).
                """
                # Wh chunks in [j, d] layout (+ ones column) for the aggregation
                whx = perlayer.tile([128, NCH * (D + 1)], F16, tag="whx")
                nc.vector.memset(whx[:], 1.0)
                whx3 = whx.rearrange("p (c w) -> p c w", w=D + 1)
                for g in range(8):
                    ps = psB.tile([128, 512], F32, tag="psB")
                    for cc in range(8):
                        c = g * 8 + cc
                        nc.tensor.matmul(
                            ps[:, cc * D : (cc + 1) * D],
                            lhsT=xa_sb[:, c * 128 : (c + 1) * 128],
                            rhs=wtb_sb[:],
                            start=True,
                            stop=True,
                        )
                    nc.scalar.activation(
                        whx3[:, g * 8 : (g + 1) * 8, 0:D], ps[:], AF.Copy
                    )

                # dst contribution, per-partition per chunk: [128, 64] f32
                dstc = perlayer.tile([128, NCH], F32, tag="dstc")
                for g in range(8):
                    psd = psB.tile([128, 8], F32, tag="psD")
                    for cc in range(8):
                        c = g * 8 + cc
                        nc.tensor.matmul(
                            psd[:, cc : cc + 1],
                            lhsT=xa_sb[:, c * 128 : (c + 1) * 128],
                            rhs=wdst_sb[:],
                            start=True,
                            stop=True,
                        )
                    nc.scalar.activation(dstc[:, g * 8 : (g + 1) * 8], psd[:], AF.Copy)

                # src contribution for this core's rows: [1, 1024] -> bcast
                srcf = perlayer.tile([1, R], F32, tag="srcf")
                for h in range(2):
                    pss = psB.tile([1, 512], F32, tag="psB")
                    nc.tensor.matmul(
                        pss[:],
                        lhsT=wsrc_sb[:],
                        rhs=xm_sb[:, h * 512 : (h + 1) * 512],
                        start=True,
                        stop=True,
                    )
                    nc.scalar.activation(
                        srcf[:, h * 512 : (h + 1) * 512], pss[:], AF.Copy
                    )
                srcrep = perlayer.tile([128, R], F16, tag="srcrep")
                for h in range(2):
                    psb = psB.tile([128, 512], F32, tag="psB")
                    nc.tensor.matmul(
                        psb[:], lhsT=ones128[:], rhs=srcf[:, h * 512 : (h + 1) * 512],
                        start=True, stop=True,
                    )
                    nc.scalar.activation(
                        srcrep[:, h * 512 : (h + 1) * 512], psb[:], AF.Copy
                    )

                # main loop over the 64 source-node chunks, processed in pairs
                # lrelu(s) = 0.5*s4 + 2*relu(s4)  (s4 = 0.4*s, additive mask in)
                #          = s4x + 4*relu(s4x)    (s4x = 0.5*s4)
                agg0 = psA.tile([D + 1, 512], F32, tag="agg0")
                agg1 = psA.tile([D + 1, 512], F32, tag="agg1")
                for cp in range(NCH // 2):
                    tp = work.tile([128, 2 * R], F16, tag="tv")
                    for ci in range(2):
                        c = cp * 2 + ci
                        adjc = work.tile([128, R], F16, tag="adj", bufs=4)
                        nc.sync.dma_start(adjc[:], adjm[c * 128 : (c + 1) * 128, :])
                        u = work.tile([128, R], F16, tag="u")
                        nc.vector.tensor_tensor(u[:], adjc[:], srcrep[:], AOP.add)
                        s4x = work.tile([128, R], F16, tag="s4x")
                        nc.vector.tensor_scalar(
                            s4x[:], u[:], dstc[:, c : c + 1], 0.5,
                            op0=AOP.add, op1=AOP.mult,
                        )
                        r2 = work.tile([128, R], F16, tag="r2")
                        nc.vector.tensor_scalar(
                            r2[:], s4x[:], 0.0, 4.0, op0=AOP.max, op1=AOP.mult
                        )
                        eng = nc.gpsimd if (c % 4) < GPSIMD_TT_FRAC4 else nc.vector
                        eng.tensor_tensor(
                            tp[:, ci * R : (ci + 1) * R], s4x[:], r2[:], AOP.add
                        )
                    pv = work.tile([128, 2 * R], F16, tag="pv")
                    nc.scalar.activation(pv[:], tp[:], AF.Exp)
                    for ci in range(2):
                        c = cp * 2 + ci
                        nc.tensor.matmul(
                            agg0[:], lhsT=whx3[:, c, :],
                            rhs=pv[:, ci * R : ci * R + 512],
                            start=(c == 0), stop=(c == NCH - 1),
                        )
                        nc.tensor.matmul(
                            agg1[:], lhsT=whx3[:, c, :],
                            rhs=pv[:, ci * R + 512 : (ci + 1) * R],
                            start=(c == 0), stop=(c == NCH - 1),
                        )

                # normalize + relu -> xnT [65, 1024] (row 64 = ones)
                zinv = perlayer.tile([1, R], F32, tag="zinv")
                nc.vector.reciprocal(zinv[:, 0:512], agg0[D : D + 1, :])
                nc.vector.reciprocal(zinv[:, 512:1024], agg1[D : D + 1, :])
                zrep = perlayer.tile([D, R], F32, tag="zrep")
                for h in range(2):
                    psb = psB.tile([D, 512], F32, tag="psB")
                    nc.tensor.matmul(
                        psb[:], lhsT=ones128[:, 0:D],
                        rhs=zinv[:, h * 512 : (h + 1) * 512],
                        start=True, stop=True,
                    )
                    nc.scalar.activation(
                        zrep[:, h * 512 : (h + 1) * 512], psb[:], AF.Copy
                    )
                xnT = perlayer.tile([D + 1, R], F32, tag="xnT")
                nc.vector.memset(xnT[D : D + 1, :], 1.0)
                nc.vector.tensor_tensor(
                    xnT[0:D, 0:512], agg0[0:D, :], zrep[:, 0:512], AOP.mult
                )
                nc.vector.tensor_tensor(
                    xnT[0:D, 512:1024], agg1[0:D, :], zrep[:, 512:1024], AOP.mult
                )
                nc.scalar.activation(xnT[0:D, :], xnT[0:D, :], AF.Relu)
                return xnT

            # ---------------- layer 0 ----------------
            x1T = gat_layer(xTa_sb, xTm_sb, w0tb_sb, wsrc0_sb, wdst0_sb)

            # AllGather x1 shards (transposed) across the 8 cores
            bounce = dram.tile([D, R], F32)
            nc.sync.dma_start(bounce[:], x1T[0:D, :])
            gath = dram.tile([NCORES * D, R], F32, addr_space="Shared")
            nc.gpsimd.collective_compute(
                "AllGather",
                AOP.bypass,
                replica_groups=[list(range(NCORES))],
                ins=[bounce[:]],
                outs=[gath[:]],
            )
            x1g = perlayer.tile([D + 1, N], F32, tag="xg")
            nc.vector.memset(x1g[D : D + 1, :], 1.0)
            for b in range(NCORES):
                nc.sync.dma_start(
                    x1g[0:D, b * R : (b + 1) * R], gath[b * D : (b + 1) * D, :]
                )

            # ---------------- layer 1 ----------------
            x2T = gat_layer(x1g, x1T, w1tb_sb, wsrc1_sb, wdst1_sb)

            # ---------------- output linear ----------------
            outsb = const.tile([D, R], F32, tag="outsb")
            for h in range(2):
                psf = psB.tile([D, 512], F32, tag="psB")
                nc.tensor.matmul(
                    psf[:],
                    lhsT=owt_sb[:],
                    rhs=x2T[0:D, h * 512 : (h + 1) * 512],
                    start=True,
                    stop=True,
                )
                nc.scalar.activation(
                    outsb[:, h * 512 : (h + 1) * 512], psf[:], AF.Identity,
                    bias=outb_sb[:, 0:1],
                )
            nc.sync.dma_start(outT[:], outsb[:])

    nc.compile()
    return nc


def _prep_inputs(adj, user_emb, item_emb, W0_w, W0_b, a0, W1_w, W1_b, a1,
                 out_w, out_b):
    x = np.concatenate([np.asarray(user_emb), np.asarray(item_emb)], axis=0)
    x = x.astype(np.float32)
    xTa = np.concatenate([x.T, np.ones((1, N), np.float32)], axis=0)
    xTa = np.ascontiguousarray(xTa)

    adj = np.asarray(adj)
    adjm_full = ((adj - 1) * 100).astype(np.float16)  # {0, -100}, 0.4-pre-scaled

    def aug_wt(W, b):
        return np.ascontiguousarray(
            np.concatenate([W.T, b[None, :]], axis=0).astype(np.float32)
        )

    def aug_attn(W, b, avec):
        w = W.T.astype(np.float64) @ avec.astype(np.float64).reshape(D, 1)
        c = float(b.astype(np.float64) @ avec.astype(np.float64).reshape(D))
        v = np.concatenate([w, [[c]]], axis=0) * 0.4
        return np.ascontiguousarray(v.astype(np.float32))

    W0_w, W0_b = np.asarray(W0_w, np.float32), np.asarray(W0_b, np.float32)
    W1_w, W1_b = np.asarray(W1_w, np.float32), np.asarray(W1_b, np.float32)
    a0, a1 = np.asarray(a0, np.float32), np.asarray(a1, np.float32)
    out_w, out_b = np.asarray(out_w, np.float32), np.asarray(out_b, np.float32)

    shared = {
        "xTa": xTa,
        "w0tb": aug_wt(W0_w, W0_b),
        "w1tb": aug_wt(W1_w, W1_b),
        "wsrc0": aug_attn(W0_w, W0_b, a0[:D]),
        "wdst0": aug_attn(W0_w, W0_b, a0[D:]),
        "wsrc1": aug_attn(W1_w, W1_b, a1[:D]),
        "wdst1": aug_attn(W1_w, W1_b, a1[D:]),
        "owt": np.ascontiguousarray(out_w.T.astype(np.float32)),
        "outb": np.ascontiguousarray(out_b.reshape(D, 1).astype(np.float32)),
    }
    in_maps = []
    for k in range(NCORES):
        m = dict(shared)
        m["adjm"] = np.ascontiguousarray(adjm_full[k * R : (k + 1) * R, :].T)
        m["xTm"] = np.ascontiguousarray(xTa[:, k * R : (k + 1) * R])
        in_maps.append(m)
    return in_maps


_NC_CACHE = {}


def run(inputs: dict, trace: bool = False):
    if "nc" not in _NC_CACHE:
        _NC_CACHE["nc"] = _build_bass()
    nc = _NC_CACHE["nc"]
    in_maps = _prep_inputs(**inputs)
    res = run_bass_kernel_spmd(nc, in_maps, list(range(NCORES)), trace=trace)
    shards = [res.results[k]["outT"].T for k in range(NCORES)]
    full = np.concatenate(shards, axis=0).astype(np.float32)
    return (full[:NU], full[NU:]), res


def kernel(**inputs):
    out, _ = run(inputs, trace=False)
    return out
